# revision 12
# baseline (speedup 1.0000x reference)
"""DescriptorLoss kernel for Trainium2 (8 NeuronCores, SPMD data-parallel).

Math:
    d[b,ij,kl] = sum_c desc0[b,c,ij] * desc1[b,c,kl]
    loss = mean(where(mask, 250*relu(1 - d), relu(d - 0.2)))

Per core (shard = (batch, i-slab) -> 1024 ij rows x 4096 kl cols):
    PE: d' = 5*d via bf16 matmuls into PSUM fp32 (lhsT = 5*desc0 slab).
    With hinges at 1 and 5 (exact in fp8e5m2):
      relu(d-0.2) = (max(d',1) - 1)/5,   relu(1-d) = (5 - min(d',5))/5
    Each masked hinge-sum is ONE fused DVE scalar_tensor_tensor op reading
    d' straight from PSUM (PSUM-source DVE ops dodge the 2.3x SBUF-source
    silicon errata), with clamp-encoded fp8 masks and fused accumulation:
      r1 = min(max(d',1), X)  X = 1  if m else  C  -> acc1 += sum(r1)
      r2 = max(min(d',5), Y)  Y = -C if m else  5  -> acc2 += sum(r2)
    With N elements per core:
      sum((1-m)*relu(d-0.2)) = (acc1 - N)/5
      sum(m*relu(1-d))       = (5N - acc2)/5
      S_core = (acc1 - 250*acc2 + 1249*N)/5
    loss = sum_cores(S_core) / (B*H*W*H*W)
All clamp constants (1, 5, +-2048) are exact in float8_e5m2, so the mask
encodings add zero error beyond bf16 matmul rounding (~2e-6 relative).
"""

import numpy as np
import ml_dtypes

import concourse.bass as bass
import concourse.bacc as bacc
import concourse.mybir as mybir
import concourse.tile as tile
from concourse.bass_utils import run_bass_kernel_spmd

B, D, H, W = 2, 128, 64, 64
N_CORES = 8
IJ = H * W               # 4096
ROWS_PER_CORE = IJ // 4  # 1024
N_PER_CORE = ROWS_PER_CORE * IJ
G = ROWS_PER_CORE // 128  # 8 row groups of 128
KTILE = 2048
KT = IJ // KTILE          # 2 kl chunks per group
N_CHUNKS = G * KT         # 16
CLAMP = 2048.0

_cached = {}


def _build_program():
    nc = bacc.Bacc("TRN2")
    f32 = mybir.dt.float32
    bf16 = mybir.dt.bfloat16
    f8 = mybir.dt.float8e5
    Alu = mybir.AluOpType

    a5 = nc.declare_dram_parameter("a5", [D, ROWS_PER_CORE], bf16, isOutput=False)
    bm = nc.declare_dram_parameter("bm", [D, IJ], bf16, isOutput=False)
    x8 = nc.declare_dram_parameter("x8", [ROWS_PER_CORE, IJ], f8, isOutput=False)
    y8 = nc.declare_dram_parameter("y8", [ROWS_PER_CORE, IJ], f8, isOutput=False)
    acc1_out = nc.declare_dram_parameter("acc1", [128, N_CHUNKS], f32, isOutput=True)
    acc2_out = nc.declare_dram_parameter("acc2", [128, N_CHUNKS], f32, isOutput=True)

    with tile.TileContext(nc) as tc:
        with (
            tc.tile_pool(name="desc", bufs=1) as desc_pool,
            tc.tile_pool(name="mask", bufs=4) as mask_pool,
            tc.tile_pool(name="scr", bufs=4) as scr_pool,
            tc.tile_pool(name="accs", bufs=1) as acc_pool,
            tc.tile_pool(name="psd", bufs=2, space="PSUM") as psum_pool,
        ):
            a_t = desc_pool.tile([D, ROWS_PER_CORE], bf16, tag="a")
            b_t = desc_pool.tile([D, IJ], bf16, tag="b")
            # sliced so the first chunk's matmuls unblock early
            nc.sync.dma_start(a_t[:, :128], a5[:, :128])
            nc.sync.dma_start(b_t[:, :KTILE], bm[:, :KTILE])
            nc.sync.dma_start(a_t[:, 128:], a5[:, 128:])
            nc.sync.dma_start(b_t[:, KTILE:], bm[:, KTILE:])

            acc1_t = acc_pool.tile([128, N_CHUNKS], f32, tag="acc1")
            acc2_t = acc_pool.tile([128, N_CHUNKS], f32, tag="acc2")

            for cid in range(N_CHUNKS):
                g, k = cid // KT, cid % KT
                rs = slice(g * 128, (g + 1) * 128)
                ks = slice(k * KTILE, (k + 1) * KTILE)

                xm_t = mask_pool.tile([128, KTILE], f8, tag="x8")
                ym_t = mask_pool.tile([128, KTILE], f8, tag="y8")
                nc.gpsimd.dma_start(xm_t[:], x8[rs, ks])
                nc.gpsimd.dma_start(ym_t[:], y8[rs, ks])

                psum_d = psum_pool.tile([128, KTILE], f32, tag="d")
                for h in range(KTILE // 512):
                    cs = slice(ks.start + h * 512, ks.start + (h + 1) * 512)
                    nc.tensor.matmul(
                        psum_d[:, h * 512:(h + 1) * 512],
                        a_t[:, rs], b_t[:, cs],
                        start=True, stop=True,
                    )

                scr1 = scr_pool.tile([128, KTILE], bf16, tag="scr")
                scr2 = scr_pool.tile([128, KTILE], bf16, tag="scr")
                nc.vector.scalar_tensor_tensor(
                    scr1[:], psum_d[:], 1.0, xm_t[:],
                    op0=Alu.max, op1=Alu.min,
                    accum_out=acc1_t[:, cid:cid + 1],
                )
                nc.vector.scalar_tensor_tensor(
                    scr2[:], psum_d[:], 5.0, ym_t[:],
                    op0=Alu.min, op1=Alu.max,
                    accum_out=acc2_t[:, cid:cid + 1],
                )

            nc.sync.dma_start(acc1_out[:], acc1_t[:])
            nc.sync.dma_start(acc2_out[:], acc2_t[:])

    nc.finalize()
    return nc


def _prep_inputs(descriptors_0, descriptors_1, similarity_mask):
    d0 = np.asarray(descriptors_0, dtype=np.float32)
    d1 = np.asarray(descriptors_1, dtype=np.float32)
    mkv = np.asarray(similarity_mask)
    C = np.float32(CLAMP)
    in_maps = []
    for c in range(N_CORES):
        b = c >> 2
        isl = (c & 3) * 16
        a5 = (d0[b].reshape(D, IJ)[:, isl * W:(isl + 16) * W] * np.float32(5.0)).astype(
            ml_dtypes.bfloat16
        )
        bmv = d1[b].reshape(D, IJ).astype(ml_dtypes.bfloat16)
        m = mkv[b, isl:isl + 16].reshape(ROWS_PER_CORE, IJ)
        in_maps.append(
            {
                "a5": np.ascontiguousarray(a5),
                "bm": np.ascontiguousarray(bmv),
                "x8": np.where(m, np.float32(1.0), C).astype(ml_dtypes.float8_e5m2),
                "y8": np.where(m, -C, np.float32(5.0)).astype(ml_dtypes.float8_e5m2),
            }
        )
    return in_maps


def _run(in_maps, **kwargs):
    if "nc" not in _cached:
        _cached["nc"] = _build_program()
    return run_bass_kernel_spmd(_cached["nc"], in_maps, list(range(N_CORES)), **kwargs)


def _combine(results):
    total = 0.0
    for r in results:
        acc1 = r["acc1"].astype(np.float64).sum()
        acc2 = r["acc2"].astype(np.float64).sum()
        total += (acc1 - 250.0 * acc2 + 1249.0 * N_PER_CORE) / 5.0
    return np.float32(total / float(B * IJ * IJ))


def kernel(descriptors_0, descriptors_1, similarity_mask):
    in_maps = _prep_inputs(descriptors_0, descriptors_1, similarity_mask)
    res = _run(in_maps)
    return _combine(res.results)


# revision 13
# speedup vs baseline: 1.0137x; 1.0137x over previous
"""DescriptorLoss kernel for Trainium2 (8 NeuronCores, SPMD data-parallel).

Math:
    d[b,ij,kl] = sum_c desc0[b,c,ij] * desc1[b,c,kl]
    loss = mean(where(mask, 250*relu(1 - d), relu(d - 0.2)))

Per core (shard = (batch, i-slab) -> 1024 ij rows x 4096 kl cols):
    PE: d' = 5*d via bf16 matmuls into PSUM fp32 (lhsT = 5*desc0 slab).
    With hinges at 1 and 5 (exact in fp8e5m2):
      relu(d-0.2) = (max(d',1) - 1)/5,   relu(1-d) = (5 - min(d',5))/5
    Each masked hinge-sum is ONE fused DVE scalar_tensor_tensor op reading
    d' straight from PSUM (PSUM-source DVE ops dodge the 2.3x SBUF-source
    silicon errata), with clamp-encoded fp8 masks and fused accumulation:
      r1 = min(max(d',1), X)  X = 1  if m else  C  -> acc1 += sum(r1)
      r2 = max(min(d',5), Y)  Y = -C if m else  5  -> acc2 += sum(r2)
    With N elements per core:
      sum((1-m)*relu(d-0.2)) = (acc1 - N)/5
      sum(m*relu(1-d))       = (5N - acc2)/5
      S_core = (acc1 - 250*acc2 + 1249*N)/5
    loss = sum_cores(S_core) / (B*H*W*H*W)
All clamp constants (1, 5, +-2048) are exact in float8_e5m2, so the mask
encodings add zero error beyond bf16 matmul rounding (~2e-6 relative).
"""

import numpy as np
import ml_dtypes

import concourse.bass as bass
import concourse.bacc as bacc
import concourse.mybir as mybir
import concourse.tile as tile
from concourse.bass_utils import run_bass_kernel_spmd

B, D, H, W = 2, 128, 64, 64
N_CORES = 8
IJ = H * W               # 4096
ROWS_PER_CORE = IJ // 4  # 1024
N_PER_CORE = ROWS_PER_CORE * IJ
G = ROWS_PER_CORE // 128  # 8 row groups of 128
KTILE = 2048
KT = IJ // KTILE          # 2 kl chunks per group
N_CHUNKS = G * KT         # 16
CLAMP = 2048.0

_cached = {}


def _build_program():
    nc = bacc.Bacc("TRN2")
    f32 = mybir.dt.float32
    bf16 = mybir.dt.bfloat16
    f8 = mybir.dt.float8e5
    Alu = mybir.AluOpType

    a5 = nc.declare_dram_parameter("a5", [D, ROWS_PER_CORE], bf16, isOutput=False)
    bm = nc.declare_dram_parameter("bm", [D, IJ], bf16, isOutput=False)
    x8 = nc.declare_dram_parameter("x8", [ROWS_PER_CORE, IJ], f8, isOutput=False)
    y8 = nc.declare_dram_parameter("y8", [ROWS_PER_CORE, IJ], f8, isOutput=False)
    acc1_out = nc.declare_dram_parameter("acc1", [128, N_CHUNKS], f32, isOutput=True)
    acc2_out = nc.declare_dram_parameter("acc2", [128, N_CHUNKS], f32, isOutput=True)

    with tile.TileContext(nc) as tc:
        with (
            tc.tile_pool(name="desc", bufs=1) as desc_pool,
            tc.tile_pool(name="mask", bufs=4) as mask_pool,
            tc.tile_pool(name="scr", bufs=4) as scr_pool,
            tc.tile_pool(name="accs", bufs=1) as acc_pool,
            tc.tile_pool(name="psd", bufs=2, space="PSUM") as psum_pool,
        ):
            a_t = desc_pool.tile([D, ROWS_PER_CORE], bf16, tag="a")
            b_t = desc_pool.tile([D, IJ], bf16, tag="b")
            # sliced so the first chunk's matmuls unblock early
            nc.sync.dma_start(a_t[:, :128], a5[:, :128])
            nc.sync.dma_start(b_t[:, :KTILE], bm[:, :KTILE])
            nc.sync.dma_start(a_t[:, 128:], a5[:, 128:])
            nc.sync.dma_start(b_t[:, KTILE:], bm[:, KTILE:])

            acc1_t = acc_pool.tile([128, N_CHUNKS], f32, tag="acc1")
            acc2_t = acc_pool.tile([128, N_CHUNKS], f32, tag="acc2")

            for cid in range(N_CHUNKS):
                g, k = cid // KT, cid % KT
                rs = slice(g * 128, (g + 1) * 128)
                ks = slice(k * KTILE, (k + 1) * KTILE)

                xm_t = mask_pool.tile([128, KTILE], f8, tag="x8")
                ym_t = mask_pool.tile([128, KTILE], f8, tag="y8")
                nc.sync.dma_start(xm_t[:], x8[rs, ks])
                nc.sync.dma_start(ym_t[:], y8[rs, ks])

                psum_d = psum_pool.tile([128, KTILE], f32, tag="d")
                for h in range(KTILE // 512):
                    cs = slice(ks.start + h * 512, ks.start + (h + 1) * 512)
                    nc.tensor.matmul(
                        psum_d[:, h * 512:(h + 1) * 512],
                        a_t[:, rs], b_t[:, cs],
                        start=True, stop=True,
                    )

                scr1 = scr_pool.tile([128, KTILE], bf16, tag="scr")
                scr2 = scr_pool.tile([128, KTILE], bf16, tag="scr")
                nc.vector.scalar_tensor_tensor(
                    scr1[:], psum_d[:], 1.0, xm_t[:],
                    op0=Alu.max, op1=Alu.min,
                    accum_out=acc1_t[:, cid:cid + 1],
                )
                nc.vector.scalar_tensor_tensor(
                    scr2[:], psum_d[:], 5.0, ym_t[:],
                    op0=Alu.min, op1=Alu.max,
                    accum_out=acc2_t[:, cid:cid + 1],
                )

            nc.sync.dma_start(acc1_out[:], acc1_t[:])
            nc.sync.dma_start(acc2_out[:], acc2_t[:])

    nc.finalize()
    return nc


def _prep_inputs(descriptors_0, descriptors_1, similarity_mask):
    d0 = np.asarray(descriptors_0, dtype=np.float32)
    d1 = np.asarray(descriptors_1, dtype=np.float32)
    mkv = np.asarray(similarity_mask)
    C = np.float32(CLAMP)
    in_maps = []
    for c in range(N_CORES):
        b = c >> 2
        isl = (c & 3) * 16
        a5 = (d0[b].reshape(D, IJ)[:, isl * W:(isl + 16) * W] * np.float32(5.0)).astype(
            ml_dtypes.bfloat16
        )
        bmv = d1[b].reshape(D, IJ).astype(ml_dtypes.bfloat16)
        m = mkv[b, isl:isl + 16].reshape(ROWS_PER_CORE, IJ)
        in_maps.append(
            {
                "a5": np.ascontiguousarray(a5),
                "bm": np.ascontiguousarray(bmv),
                "x8": np.where(m, np.float32(1.0), C).astype(ml_dtypes.float8_e5m2),
                "y8": np.where(m, -C, np.float32(5.0)).astype(ml_dtypes.float8_e5m2),
            }
        )
    return in_maps


def _run(in_maps, **kwargs):
    if "nc" not in _cached:
        _cached["nc"] = _build_program()
    return run_bass_kernel_spmd(_cached["nc"], in_maps, list(range(N_CORES)), **kwargs)


def _combine(results):
    total = 0.0
    for r in results:
        acc1 = r["acc1"].astype(np.float64).sum()
        acc2 = r["acc2"].astype(np.float64).sum()
        total += (acc1 - 250.0 * acc2 + 1249.0 * N_PER_CORE) / 5.0
    return np.float32(total / float(B * IJ * IJ))


def kernel(descriptors_0, descriptors_1, similarity_mask):
    in_maps = _prep_inputs(descriptors_0, descriptors_1, similarity_mask)
    res = _run(in_maps)
    return _combine(res.results)


# revision 14
# speedup vs baseline: 1.0184x; 1.0046x over previous
"""DescriptorLoss kernel for Trainium2 (8 NeuronCores, SPMD data-parallel).

Math:
    d[b,ij,kl] = sum_c desc0[b,c,ij] * desc1[b,c,kl]
    loss = mean(where(mask, 250*relu(1 - d), relu(d - 0.2)))

Per core (shard = (batch, i-slab) -> 1024 ij rows x 4096 kl cols):
    PE: d' = 5*d via bf16 matmuls into PSUM fp32 (lhsT = 5*desc0 slab).
    With hinges at 1 and 5 (exact in fp8e5m2):
      relu(d-0.2) = (max(d',1) - 1)/5,   relu(1-d) = (5 - min(d',5))/5
    Each masked hinge-sum is ONE fused DVE scalar_tensor_tensor op reading
    d' straight from PSUM (PSUM-source DVE ops dodge the 2.3x SBUF-source
    silicon errata), with clamp-encoded fp8 masks and fused accumulation:
      r1 = min(max(d',1), X)  X = 1  if m else  C  -> acc1 += sum(r1)
      r2 = max(min(d',5), Y)  Y = -C if m else  5  -> acc2 += sum(r2)
    With N elements per core:
      sum((1-m)*relu(d-0.2)) = (acc1 - N)/5
      sum(m*relu(1-d))       = (5N - acc2)/5
      S_core = (acc1 - 250*acc2 + 1249*N)/5
    loss = sum_cores(S_core) / (B*H*W*H*W)
All clamp constants (1, 5, +-2048) are exact in float8_e5m2, so the mask
encodings add zero error beyond bf16 matmul rounding (~2e-6 relative).
"""

import numpy as np
import ml_dtypes

import concourse.bass as bass
import concourse.bacc as bacc
import concourse.mybir as mybir
import concourse.tile as tile
from concourse.bass_utils import run_bass_kernel_spmd

B, D, H, W = 2, 128, 64, 64
N_CORES = 8
IJ = H * W               # 4096
ROWS_PER_CORE = IJ // 4  # 1024
N_PER_CORE = ROWS_PER_CORE * IJ
G = ROWS_PER_CORE // 128  # 8 row groups of 128
KTILE = 2048
KT = IJ // KTILE          # 2 kl chunks per group
N_CHUNKS = G * KT         # 16
CLAMP = 2048.0

_cached = {}


def _build_program():
    nc = bacc.Bacc("TRN2")
    f32 = mybir.dt.float32
    bf16 = mybir.dt.bfloat16
    f8 = mybir.dt.float8e5
    Alu = mybir.AluOpType

    a5 = nc.declare_dram_parameter("a5", [D, ROWS_PER_CORE], bf16, isOutput=False)
    bm = nc.declare_dram_parameter("bm", [D, IJ], bf16, isOutput=False)
    x8 = nc.declare_dram_parameter("x8", [ROWS_PER_CORE, IJ], f8, isOutput=False)
    y8 = nc.declare_dram_parameter("y8", [ROWS_PER_CORE, IJ], f8, isOutput=False)
    acc1_out = nc.declare_dram_parameter("acc1", [128, N_CHUNKS], f32, isOutput=True)
    acc2_out = nc.declare_dram_parameter("acc2", [128, N_CHUNKS], f32, isOutput=True)

    with tile.TileContext(nc) as tc:
        with (
            tc.tile_pool(name="desc", bufs=1) as desc_pool,
            tc.tile_pool(name="mask", bufs=4) as mask_pool,
            tc.tile_pool(name="scr", bufs=4) as scr_pool,
            tc.tile_pool(name="accs", bufs=1) as acc_pool,
            tc.tile_pool(name="psd", bufs=2, space="PSUM") as psum_pool,
        ):
            a_t = desc_pool.tile([D, ROWS_PER_CORE], bf16, tag="a")
            b_t = desc_pool.tile([D, IJ], bf16, tag="b")
            # sliced so the first chunk's matmuls unblock early; the
            # remainder is issued after chunk 0's mask DMAs
            nc.sync.dma_start(a_t[:, :128], a5[:, :128])
            nc.sync.dma_start(b_t[:, :KTILE], bm[:, :KTILE])

            acc1_t = acc_pool.tile([128, N_CHUNKS], f32, tag="acc1")
            acc2_t = acc_pool.tile([128, N_CHUNKS], f32, tag="acc2")

            for cid in range(N_CHUNKS):
                if cid == 1:
                    nc.sync.dma_start(a_t[:, 128:], a5[:, 128:])
                    nc.sync.dma_start(b_t[:, KTILE:], bm[:, KTILE:])
                g, k = cid // KT, cid % KT
                rs = slice(g * 128, (g + 1) * 128)
                ks = slice(k * KTILE, (k + 1) * KTILE)

                xm_t = mask_pool.tile([128, KTILE], f8, tag="x8")
                ym_t = mask_pool.tile([128, KTILE], f8, tag="y8")
                nc.sync.dma_start(xm_t[:], x8[rs, ks])
                nc.sync.dma_start(ym_t[:], y8[rs, ks])

                psum_d = psum_pool.tile([128, KTILE], f32, tag="d")
                for h in range(KTILE // 512):
                    cs = slice(ks.start + h * 512, ks.start + (h + 1) * 512)
                    nc.tensor.matmul(
                        psum_d[:, h * 512:(h + 1) * 512],
                        a_t[:, rs], b_t[:, cs],
                        start=True, stop=True,
                    )

                scr1 = scr_pool.tile([128, KTILE], bf16, tag="scr")
                scr2 = scr_pool.tile([128, KTILE], bf16, tag="scr")
                nc.vector.scalar_tensor_tensor(
                    scr1[:], psum_d[:], 1.0, xm_t[:],
                    op0=Alu.max, op1=Alu.min,
                    accum_out=acc1_t[:, cid:cid + 1],
                )
                nc.vector.scalar_tensor_tensor(
                    scr2[:], psum_d[:], 5.0, ym_t[:],
                    op0=Alu.min, op1=Alu.max,
                    accum_out=acc2_t[:, cid:cid + 1],
                )

            nc.sync.dma_start(acc1_out[:], acc1_t[:])
            nc.sync.dma_start(acc2_out[:], acc2_t[:])

    nc.finalize()
    return nc


def _prep_inputs(descriptors_0, descriptors_1, similarity_mask):
    d0 = np.asarray(descriptors_0, dtype=np.float32)
    d1 = np.asarray(descriptors_1, dtype=np.float32)
    mkv = np.asarray(similarity_mask)
    C = np.float32(CLAMP)
    in_maps = []
    for c in range(N_CORES):
        b = c >> 2
        isl = (c & 3) * 16
        a5 = (d0[b].reshape(D, IJ)[:, isl * W:(isl + 16) * W] * np.float32(5.0)).astype(
            ml_dtypes.bfloat16
        )
        bmv = d1[b].reshape(D, IJ).astype(ml_dtypes.bfloat16)
        m = mkv[b, isl:isl + 16].reshape(ROWS_PER_CORE, IJ)
        in_maps.append(
            {
                "a5": np.ascontiguousarray(a5),
                "bm": np.ascontiguousarray(bmv),
                "x8": np.where(m, np.float32(1.0), C).astype(ml_dtypes.float8_e5m2),
                "y8": np.where(m, -C, np.float32(5.0)).astype(ml_dtypes.float8_e5m2),
            }
        )
    return in_maps


def _run(in_maps, **kwargs):
    if "nc" not in _cached:
        _cached["nc"] = _build_program()
    return run_bass_kernel_spmd(_cached["nc"], in_maps, list(range(N_CORES)), **kwargs)


def _combine(results):
    total = 0.0
    for r in results:
        acc1 = r["acc1"].astype(np.float64).sum()
        acc2 = r["acc2"].astype(np.float64).sum()
        total += (acc1 - 250.0 * acc2 + 1249.0 * N_PER_CORE) / 5.0
    return np.float32(total / float(B * IJ * IJ))


def kernel(descriptors_0, descriptors_1, similarity_mask):
    in_maps = _prep_inputs(descriptors_0, descriptors_1, similarity_mask)
    res = _run(in_maps)
    return _combine(res.results)
